# revision 29
# baseline (speedup 1.0000x reference)
"""Causal self-attention (B=8, T=1024, C=768, H=12) on 8 Trainium2 NeuronCores.

Sharding: data parallel — one batch element per core, no collectives.

Per-core Bass/Tile kernel, all matmul operands bf16 (1 cyc/row at any N;
fp32 PSUM accumulate; host pre-quantizes weights + x to bf16):
  Q^T, K^T = Wqkv.T @ x^T                  (features on partitions)
  V = x^T.T @ Wv                           (natural layout, per-head + ones col)
  per head: S^T = K_h^T.T @ Q_h^T          (k on partitions, exact causal chunks)
    causal mask on the diagonal 128x128 block added on PSUM via a bf16
    matmul (negmask^T.T @ I), then ACT exp(s/8) -> ragged P^T
    O'^T = [V_h | 1].T @ P^T               (row 64 = softmax denominator)
    normalize: DVE reciprocal + tiny DMA to partition 0 + GPSIMD
    partition_broadcast + DVE multiply (odd heads take an SBUF->SBUF DMA to
    reach partitions 64..127 — DVE lanes are partition-locked)
  y = attn'^T.T @ Wp + bias
"""
import sys
from contextlib import ExitStack

import numpy as np

for _p in ("/opt/trn_rl_repo", "/root/.axon_site/_ro/trn_rl_repo"):
    if _p not in sys.path:
        sys.path.insert(0, _p)

import concourse.bass as bass  # noqa: E402
import concourse.mybir as mybir  # noqa: E402

F32 = mybir.dt.float32
BF16 = mybir.dt.bfloat16
AF = mybir.ActivationFunctionType
OP = mybir.AluOpType

B, T, C, H, D = 8, 1024, 768, 12, 64
N_CORES = 8


def _chunks_512(a, b):
    out = []
    while a < b:
        nxt = min((a // 512 + 1) * 512, b)
        out.append((a, nxt))
        a = nxt
    return out


def _emit_attention(tc, io):
    nc = tc.nc
    NT = T // 128
    NC = C // 128

    off = [0] * (NT + 1)
    for i in range(NT):
        off[i + 1] = off[i] + (T - 128 * i)
    PTW = off[NT]

    with ExitStack() as stack:
        persist = stack.enter_context(tc.tile_pool(name="persist", bufs=1))
        consts = stack.enter_context(tc.tile_pool(name="consts", bufs=1))
        negmT = consts.tile([128, 128], BF16, tag="negmT")
        idb = consts.tile([128, 128], BF16, tag="idb")
        nc.sync.dma_start(negmT[:], io["negmaskT16"])
        nc.sync.dma_start(idb[:], io["identity16"])
        bqt_sb = consts.tile([128, 3 * C // 128], F32, tag="bqt")
        nc.sync.dma_start(bqt_sb[:], io["bqkvT"])
        bb_sb = consts.tile([128, 2 * C], F32, tag="bb")
        nc.sync.dma_start(bb_sb[:], io["bias_bcast"])
        ones_sb = consts.tile([128, 128], F32, tag="ones_sb")
        nc.sync.dma_start(ones_sb[:], io["ones"])

        qt = persist.tile([128, NC, T], BF16, tag="qt")
        kt_ = persist.tile([128, NC, T], BF16, tag="kt")
        vp = persist.tile([128, NT, H, D + 1], BF16, tag="vp")
        attnT = persist.tile([128, NC, T], BF16, tag="attnT")
        wpp = persist.tile([128, NC, C], BF16, tag="wpp")
        x1t = persist.tile([128, NC, T], BF16, tag="x1t")
        wq_sb = persist.tile([128, NC, 3 * C], BF16, tag="wq")

        psu = stack.enter_context(tc.tile_pool(name="psu", bufs=3, space="PSUM"))
        ps2b = stack.enter_context(tc.tile_pool(name="ps2b", bufs=2, space="PSUM"))
        p2 = stack.enter_context(tc.tile_pool(name="p2", bufs=2))
        p2o = stack.enter_context(tc.tile_pool(name="p2o", bufs=3))
        p2p = stack.enter_context(tc.tile_pool(name="p2p", bufs=2))

        nc.vector.tensor_copy(
            vp[:, :, :, 64],
            ones_sb[:, 0:NT * H].rearrange("p (t h) -> p t h", h=H))

        # DMA order = PE consumption order: group ft consumes Q cols
        # [ft*128,(ft+1)*128), K cols C+same, V cols 2C+same. x is needed
        # in full by the very first matmul.
        def load_wq(w0, w1):
            for kt in range(NC):
                nc.sync.dma_start(
                    wq_sb[:, kt, w0:w1],
                    io["wqkv"][kt * 128:(kt + 1) * 128, w0:w1])

        def vcol(ft):
            return (2 * C + ft * 128, 2 * C + (ft + 1) * 128)

        load_wq(0, 256)
        load_wq(C, C + 256)
        load_wq(*vcol(0))
        for c in range(NC):
            nc.sync.dma_start(x1t[:, c, :],
                              io["xT"][c * 128:(c + 1) * 128, :])
        load_wq(*vcol(1))
        for mp in (1, 2):
            load_wq(mp * 256, mp * 256 + 256)
            load_wq(C + mp * 256, C + mp * 256 + 256)
            load_wq(*vcol(2 * mp))
            load_wq(*vcol(2 * mp + 1))
        for kt in range(NC):
            nc.sync.dma_start(wpp[:, kt, :],
                              io["wp"][kt * 128:(kt + 1) * 128, :])

        onrm = None
        pending_tr = None

        def flush_tr():
            nonlocal pending_tr
            if pending_tr is None:
                return
            onrm_p, ft_p = pending_tr
            pending_tr = None
            # transpose [q, (pair, d)] -> [(pair, d), q] feature-major
            ps_t = psu.tile([128, T], F32, tag="ps")
            ps_tb = ps_t.bitcast(BF16)
            for qt_ in range(NT):
                nc.tensor.transpose(ps_tb[:, qt_ * 128:(qt_ + 1) * 128],
                                    onrm_p[:, qt_, :, :], idb[:])
            nc.vector.tensor_copy(attnT[:, ft_p, :], ps_tb[:, 0:T])

        # ---- QKV for feature tile ft, split into ~1.3us PE units so they
        # can interleave with S strips of the previous group ----
        def qkv_units(ft):
            # each unit is self-contained (PSUM alloc -> matmuls -> DVE):
            # splitting a tile's lifetime across interleaved strips would
            # deadlock the in-order PE queue on the pool ring
            if ft >= NC:
                return []

            def qk_u(m, dest):
                ps = psu.tile([128, T], F32, tag="ps")
                for (a, b) in _chunks_512(0, T):
                    for kt in range(NC):
                        nc.tensor.matmul(
                            ps[:, a:b], wq_sb[:, kt, m * 128:(m + 1) * 128],
                            x1t[:, kt, a:b], start=(kt == 0),
                            stop=(kt == NC - 1))
                # bias is per-partition here: fuse it into the copy
                nc.vector.tensor_scalar_add(dest[:, ft, :], ps[:],
                                            bqt_sb[:, m:m + 1])

            def v_u():
                w0, w1 = vcol(ft)
                ps = psu.tile([128, T], F32, tag="ps")
                for t in range(NT):
                    for kt in range(NC):
                        nc.tensor.matmul(
                            ps[:, t * 128:(t + 1) * 128],
                            x1t[:, kt, t * 128:(t + 1) * 128],
                            wq_sb[:, kt, w0:w1],
                            start=(kt == 0), stop=(kt == NC - 1))
                nc.vector.tensor_tensor(
                    vp[:, :, 2 * ft:2 * ft + 2, 0:D],
                    ps[:, 0:T].rearrange("p (t h d) -> p t h d", h=2, d=D),
                    bb_sb[:, ft * 128:(ft + 1) * 128]
                    .rearrange("p (h d) -> p h d", d=D)[:, None, :, :]
                    .to_broadcast([128, NT, 2, D]),
                    OP.add)

            return [lambda: qk_u(ft, qt),
                    lambda: qk_u(NC + ft, kt_),
                    v_u]

        def emit_strip(h, ft, kt, pt_sb):
            p0 = 64 * (h % 2)
            base = (kt * 128 // 512) * 512
            ps_s = psu.tile([128, T], F32, tag="ps")
            for (a, b) in _chunks_512(kt * 128, T):
                diag = a == kt * 128
                nc.tensor.matmul(
                    ps_s[:, a - base:b - base],
                    kt_[p0:p0 + 64, ft, kt * 128:(kt + 1) * 128],
                    qt[p0:p0 + 64, ft, a:b],
                    start=True, stop=not diag)
                if diag:
                    nc.tensor.matmul(ps_s[:, a - base:a - base + 128],
                                     negmT[:], idb[:],
                                     start=False, stop=True)
            # one exp per strip -> ragged P^T
            nc.scalar.activation(
                pt_sb[:, off[kt]:off[kt + 1]],
                ps_s[:, kt * 128 - base:T - base],
                AF.Exp, bias=0.0, scale=1.0 / np.sqrt(D))

        def emit_pv(h, ft, pt_sb, onrm):
            hi = h % 2  # pair row: even head -> 0, odd -> 1
            for (q0, q1) in _chunks_512(0, T):
                nq = (q1 - q0) // 128
                qb = q0 // 128
                # O = P^T.T @ [V|1]: q on partitions -> per-partition
                # denominator in column D, normalize with a DVE
                # broadcast-multiply (no partition broadcast needed)
                ps_o = ps2b.tile([128, 4, D + 1], F32, tag="ps_o")
                for i in range(nq):
                    qt0 = qb + i
                    for kt in range(qt0 + 1):
                        c0 = off[kt] + qt0 * 128 - kt * 128
                        nc.tensor.matmul(
                            ps_o[:, i, :],
                            pt_sb[:, c0:c0 + 128],
                            vp[:, kt, h, :],
                            start=(kt == 0), stop=(kt == qt0))
                dn = p2o.tile([128, 4], F32, tag="dn")
                nc.vector.reciprocal(dn[:, 0:nq], ps_o[:, 0:nq, D])
                nc.vector.tensor_tensor(
                    onrm[:, qb:qb + nq, hi, :],
                    ps_o[:, 0:nq, 0:D],
                    dn[:, 0:nq, None].to_broadcast([128, nq, D]),
                    OP.mult)

        # ---- prologue: QKV for the first feature tile, monolithic ----
        for u in qkv_units(0):
            u()

        # ---- groups: S strips of pair ft interleaved with QKV(ft+1) ----
        UNIT_AFTER = {3: 0, 8: 1, 12: 2}
        for ft in range(NC):
            h1, h0 = 2 * ft + 1, 2 * ft
            units = qkv_units(ft + 1)
            pt1 = p2.tile([128, PTW], BF16, tag="pt", name=f"pt{h1}")
            pt0 = p2.tile([128, PTW], BF16, tag="pt", name=f"pt{h0}")
            strips = [(h1, pt1, kt) for kt in range(NT)] + \
                     [(h0, pt0, kt) for kt in range(NT)]
            for si, (h, pt_sb, kt) in enumerate(strips):
                emit_strip(h, ft, kt, pt_sb)
                ui = UNIT_AFTER.get(si)
                if ui is not None and ui < len(units):
                    units[ui]()
            flush_tr()
            onrm = p2p.tile([128, NT, 2, D], BF16, tag="onrm")
            emit_pv(h1, ft, pt1, onrm)
            emit_pv(h0, ft, pt0, onrm)
            pending_tr = (onrm, ft)
        flush_tr()

        # ---------------- phase 3: projection ----------------
        with tc.tile_pool(name="p3", bufs=3) as p3:
            for t in range(NT):
                ps_y = psu.tile([128, T], F32, tag="ps")
                for (n0, n1) in _chunks_512(0, C):
                    for kt in range(NC):
                        nc.tensor.matmul(
                            ps_y[:, n0:n1], attnT[:, kt, t * 128:(t + 1) * 128],
                            wpp[:, kt, n0:n1],
                            start=(kt == 0), stop=(kt == NC - 1))
                y_sb = p3.tile([128, C], F32, tag="y_sb")
                nc.vector.tensor_tensor(y_sb[:], ps_y[:, 0:C],
                                        bb_sb[:, C:2 * C], OP.add)
                nc.sync.dma_start(io["y"][t * 128:(t + 1) * 128, :], y_sb[:])


IO_SPECS = {
    "xT": ([C, T], BF16),
    "wqkv": ([C, 3 * C], BF16),
    "bqkvT": ([128, 3 * C // 128], F32),
    "bias_bcast": ([128, 2 * C], F32),
    "wp": ([C, C], BF16),
    "ones": ([128, 128], F32),
    "negmaskT16": ([128, 128], BF16),
    "identity16": ([128, 128], BF16),
}
OUT_SPECS = {"y": ([T, C], F32)}


def build_nc():
    from concourse import bacc
    import concourse.tile as tile
    nc = bacc.Bacc("TRN2", target_bir_lowering=False, debug=False,
                   enable_asserts=True, num_devices=N_CORES)
    io = {}
    for name, (shape, dt) in IO_SPECS.items():
        io[name] = nc.dram_tensor(name, shape, dt, kind="ExternalInput").ap()
    for name, (shape, dt) in OUT_SPECS.items():
        io[name] = nc.dram_tensor(name, shape, dt, kind="ExternalOutput").ap()
    with tile.TileContext(nc) as tc:
        _emit_attention(tc, io)
    nc.compile()
    return nc


def host_consts():
    import ml_dtypes
    negmask = np.where(np.triu(np.ones((128, 128), dtype=bool)), 0.0,
                       -1e9).astype(np.float32)
    return {
        "ones": np.ones((128, 128), dtype=np.float32),
        "negmaskT16": np.ascontiguousarray(negmask.T).astype(ml_dtypes.bfloat16),
        "identity16": np.eye(128, dtype=ml_dtypes.bfloat16),
    }


_NC_CACHE = None


def _get_nc():
    global _NC_CACHE
    if _NC_CACHE is None:
        _NC_CACHE = build_nc()
    return _NC_CACHE


def make_in_maps(x, c_attn_kernel, c_attn_bias, c_proj_kernel, c_proj_bias):
    import ml_dtypes
    BF = ml_dtypes.bfloat16
    consts = host_consts()
    wqkv = np.ascontiguousarray(c_attn_kernel).astype(BF)
    bqkv = np.ascontiguousarray(c_attn_bias, dtype=np.float32)
    bqkvT = np.ascontiguousarray(bqkv.reshape(3 * C // 128, 128).T)
    wp = np.ascontiguousarray(c_proj_kernel).astype(BF)
    bp = np.ascontiguousarray(c_proj_bias, dtype=np.float32)
    bias_bcast = np.ascontiguousarray(
        np.tile(np.concatenate([bqkv[2 * C:], bp]), (128, 1)))
    in_maps = []
    for bb in range(N_CORES):
        m = {"xT": np.ascontiguousarray(np.asarray(x[bb]).T).astype(BF),
             "wqkv": wqkv, "bqkvT": bqkvT, "wp": wp,
             "bias_bcast": bias_bcast}
        m.update(consts)
        in_maps.append(m)
    return in_maps


def kernel(x, c_attn_kernel, c_attn_bias, c_proj_kernel, c_proj_bias):
    from concourse.bass_utils import run_bass_kernel_spmd
    x = np.asarray(x)
    assert x.shape == (B, T, C), x.shape
    nc = _get_nc()
    in_maps = make_in_maps(x, c_attn_kernel, c_attn_bias, c_proj_kernel,
                           c_proj_bias)
    res = run_bass_kernel_spmd(nc, in_maps, core_ids=list(range(N_CORES)))
    y = np.stack([res.results[bb]["y"] for bb in range(N_CORES)]).astype(np.float32)
    return y


# revision 34
# speedup vs baseline: 1.1286x; 1.1286x over previous
"""Causal self-attention (B=8, T=1024, C=768, H=12) on 8 Trainium2 NeuronCores.

Sharding: data parallel — one batch element per core, no collectives.

Per-core Bass/Tile kernel, all matmul operands bf16 (1 cyc/row at any N;
fp32 PSUM accumulate; host pre-quantizes weights + x to bf16):
  Q^T, K^T = Wqkv.T @ x^T                  (features on partitions)
  V = x^T.T @ Wv                           (natural layout, per-head + ones col)
  per head: S^T = K_h^T.T @ Q_h^T          (k on partitions, exact causal chunks)
    causal mask on the diagonal 128x128 block added on PSUM via a bf16
    matmul (negmask^T.T @ I), then ACT exp(s/8) -> ragged P^T
    O'^T = [V_h | 1].T @ P^T               (row 64 = softmax denominator)
    normalize: DVE reciprocal + tiny DMA to partition 0 + GPSIMD
    partition_broadcast + DVE multiply (odd heads take an SBUF->SBUF DMA to
    reach partitions 64..127 — DVE lanes are partition-locked)
  y = attn'^T.T @ Wp + bias
"""
import sys
from contextlib import ExitStack

import numpy as np

for _p in ("/opt/trn_rl_repo", "/root/.axon_site/_ro/trn_rl_repo"):
    if _p not in sys.path:
        sys.path.insert(0, _p)

import concourse.bass as bass  # noqa: E402
import concourse.mybir as mybir  # noqa: E402

F32 = mybir.dt.float32
BF16 = mybir.dt.bfloat16
AF = mybir.ActivationFunctionType
OP = mybir.AluOpType

B, T, C, H, D = 8, 1024, 768, 12, 64
N_CORES = 8


def _chunks_512(a, b):
    out = []
    while a < b:
        nxt = min((a // 512 + 1) * 512, b)
        out.append((a, nxt))
        a = nxt
    return out


def _emit_attention(tc, io):
    nc = tc.nc
    NT = T // 128
    NC = C // 128

    off = [0] * (NT + 1)
    for i in range(NT):
        off[i + 1] = off[i] + (T - 128 * i)
    PTW = off[NT]

    with ExitStack() as stack:
        persist = stack.enter_context(tc.tile_pool(name="persist", bufs=1))
        consts = stack.enter_context(tc.tile_pool(name="consts", bufs=1))
        trium = consts.tile([128, 128], BF16, tag="trium")
        idb = consts.tile([128, 128], BF16, tag="idb")
        bqt_sb = consts.tile([128, 3 * C // 128], F32, tag="bqt")
        nc.sync.dma_start(bqt_sb[:], io["bqkvT"])
        bb_sb = consts.tile([128, 2 * C], F32, tag="bb")
        ones_sb = consts.tile([128, 128], F32, tag="ones_sb")
        nc.sync.dma_start(ones_sb[:], io["ones"])

        qt = persist.tile([128, NC, T], BF16, tag="qt")
        kt_ = persist.tile([128, NC, T], BF16, tag="kt")
        vp = persist.tile([128, NT, H, D + 1], BF16, tag="vp")
        attnT = persist.tile([128, NC, T], BF16, tag="attnT")
        wpp = persist.tile([128, NC, C], BF16, tag="wpp")
        x1t = persist.tile([128, NC, T], BF16, tag="x1t")
        wq_sb = persist.tile([128, NC, 3 * C], BF16, tag="wq")

        psu = stack.enter_context(tc.tile_pool(name="psu", bufs=3, space="PSUM"))
        ps2b = stack.enter_context(tc.tile_pool(name="ps2b", bufs=2, space="PSUM"))
        p2 = stack.enter_context(tc.tile_pool(name="p2", bufs=2))
        p2o = stack.enter_context(tc.tile_pool(name="p2o", bufs=3))
        p2p = stack.enter_context(tc.tile_pool(name="p2p", bufs=2))

        nc.vector.tensor_copy(
            vp[:, :, :, 64],
            ones_sb[:, 0:NT * H].rearrange("p (t h) -> p t h", h=H))

        # DMA order = PE consumption order: group ft consumes Q cols
        # [ft*128,(ft+1)*128), K cols C+same, V cols 2C+same. x is needed
        # in full by the very first matmul.
        def load_wq(w0, w1):
            for kt in range(NC):
                nc.sync.dma_start(
                    wq_sb[:, kt, w0:w1],
                    io["wqkv"][kt * 128:(kt + 1) * 128, w0:w1])

        def vcol(ft):
            return (2 * C + ft * 128, 2 * C + (ft + 1) * 128)

        load_wq(0, 256)
        load_wq(C, C + 256)
        load_wq(*vcol(0))
        for c in range(NC):
            nc.sync.dma_start(x1t[:, c, :],
                              io["xT"][c * 128:(c + 1) * 128, :])
        # consts not needed until the first DVE/S-strip load after x
        nc.sync.dma_start(trium[:], io["triuones16"])
        nc.sync.dma_start(idb[:], io["identity16"])
        nc.sync.dma_start(bb_sb[:], io["bias_bcast"])
        load_wq(*vcol(1))
        for mp in (1, 2):
            load_wq(mp * 256, mp * 256 + 256)
            load_wq(C + mp * 256, C + mp * 256 + 256)
            load_wq(*vcol(2 * mp))
            load_wq(*vcol(2 * mp + 1))
        for kt in range(NC):
            nc.sync.dma_start(wpp[:, kt, :],
                              io["wp"][kt * 128:(kt + 1) * 128, :])

        onrm = None
        pending_tr = None

        def flush_tr():
            nonlocal pending_tr
            if pending_tr is None:
                return
            onrm_p, ft_p = pending_tr
            pending_tr = None
            # transpose [q, (pair, d)] -> [(pair, d), q] feature-major
            ps_t = psu.tile([128, T], F32, tag="ps")
            ps_tb = ps_t.bitcast(BF16)
            for qt_ in range(NT):
                nc.tensor.transpose(ps_tb[:, qt_ * 128:(qt_ + 1) * 128],
                                    onrm_p[:, qt_, :, :], idb[:])
            nc.vector.tensor_copy(attnT[:, ft_p, :], ps_tb[:, 0:T])

        # ---- QKV for feature tile ft, split into ~1.3us PE units so they
        # can interleave with S strips of the previous group ----
        def qkv_units(ft):
            # each unit is self-contained (PSUM alloc -> matmuls -> DVE):
            # splitting a tile's lifetime across interleaved strips would
            # deadlock the in-order PE queue on the pool ring
            if ft >= NC:
                return []

            def qk_u(m, dest):
                ps = psu.tile([128, T], F32, tag="ps")
                for (a, b) in _chunks_512(0, T):
                    for kt in range(NC):
                        nc.tensor.matmul(
                            ps[:, a:b], wq_sb[:, kt, m * 128:(m + 1) * 128],
                            x1t[:, kt, a:b], start=(kt == 0),
                            stop=(kt == NC - 1))
                # bias is per-partition here: fuse it into the copy
                nc.vector.tensor_scalar_add(dest[:, ft, :], ps[:],
                                            bqt_sb[:, m:m + 1])

            def v_u():
                w0, w1 = vcol(ft)
                ps = psu.tile([128, T], F32, tag="ps")
                for t in range(NT):
                    for kt in range(NC):
                        nc.tensor.matmul(
                            ps[:, t * 128:(t + 1) * 128],
                            x1t[:, kt, t * 128:(t + 1) * 128],
                            wq_sb[:, kt, w0:w1],
                            start=(kt == 0), stop=(kt == NC - 1))
                nc.vector.tensor_tensor(
                    vp[:, :, 2 * ft:2 * ft + 2, 0:D],
                    ps[:, 0:T].rearrange("p (t h d) -> p t h d", h=2, d=D),
                    bb_sb[:, ft * 128:(ft + 1) * 128]
                    .rearrange("p (h d) -> p h d", d=D)[:, None, :, :]
                    .to_broadcast([128, NT, 2, D]),
                    OP.add)

            return [lambda: qk_u(ft, qt),
                    lambda: qk_u(NC + ft, kt_),
                    v_u]

        def emit_strip(h, ft, kt, pt_sb):
            p0 = 64 * (h % 2)
            base = (kt * 128 // 512) * 512
            ps_s = psu.tile([128, T], F32, tag="ps")
            for (a, b) in _chunks_512(kt * 128, T):
                nc.tensor.matmul(
                    ps_s[:, a - base:b - base],
                    kt_[p0:p0 + 64, ft, kt * 128:(kt + 1) * 128],
                    qt[p0:p0 + 64, ft, a:b],
                    start=True, stop=True)
            # one exp per strip -> ragged P^T
            nc.scalar.activation(
                pt_sb[:, off[kt]:off[kt + 1]],
                ps_s[:, kt * 128 - base:T - base],
                AF.Exp, bias=0.0, scale=1.0 / np.sqrt(D))
            # causal mask: zero the upper triangle (k > q) of the diagonal
            # block on the idle GPSIMD engine instead of a PE mask matmul
            nc.gpsimd.tensor_tensor(
                pt_sb[:, off[kt]:off[kt] + 128],
                pt_sb[:, off[kt]:off[kt] + 128],
                trium[:], OP.mult)

        def emit_pv(h, ft, pt_sb, onrm):
            hi = h % 2  # pair row: even head -> 0, odd -> 1
            for (q0, q1) in _chunks_512(0, T):
                nq = (q1 - q0) // 128
                qb = q0 // 128
                # O = P^T.T @ [V|1]: q on partitions -> per-partition
                # denominator in column D, normalize with a DVE
                # broadcast-multiply (no partition broadcast needed)
                ps_o = ps2b.tile([128, 4, D + 1], F32, tag="ps_o")
                for i in range(nq):
                    qt0 = qb + i
                    for kt in range(qt0 + 1):
                        c0 = off[kt] + qt0 * 128 - kt * 128
                        nc.tensor.matmul(
                            ps_o[:, i, :],
                            pt_sb[:, c0:c0 + 128],
                            vp[:, kt, h, :],
                            start=(kt == 0), stop=(kt == qt0))
                dn = p2o.tile([128, 4], F32, tag="dn")
                nc.vector.reciprocal(dn[:, 0:nq], ps_o[:, 0:nq, D])
                nc.vector.tensor_tensor(
                    onrm[:, qb:qb + nq, hi, :],
                    ps_o[:, 0:nq, 0:D],
                    dn[:, 0:nq, None].to_broadcast([128, nq, D]),
                    OP.mult)

        # ---- prologue: QKV for the first feature tile, monolithic ----
        for u in qkv_units(0):
            u()

        # ---- groups: S strips of pair ft interleaved with QKV(ft+1) ----
        UNIT_AFTER = {3: 0, 8: 1, 12: 2}
        for ft in range(NC):
            h1, h0 = 2 * ft + 1, 2 * ft
            units = qkv_units(ft + 1)
            pt1 = p2.tile([128, PTW], BF16, tag="pt", name=f"pt{h1}")
            pt0 = p2.tile([128, PTW], BF16, tag="pt", name=f"pt{h0}")
            strips = [(h1, pt1, kt) for kt in range(NT)] + \
                     [(h0, pt0, kt) for kt in range(NT)]
            for si, (h, pt_sb, kt) in enumerate(strips):
                emit_strip(h, ft, kt, pt_sb)
                ui = UNIT_AFTER.get(si)
                if ui is not None and ui < len(units):
                    units[ui]()
            flush_tr()
            onrm = p2p.tile([128, NT, 2, D], BF16, tag="onrm")
            emit_pv(h1, ft, pt1, onrm)
            emit_pv(h0, ft, pt0, onrm)
            pending_tr = (onrm, ft)
        flush_tr()

        # ---------------- phase 3: projection ----------------
        with tc.tile_pool(name="p3", bufs=3) as p3:
            for t in range(NT):
                ps_y = psu.tile([128, T], F32, tag="ps")
                for (n0, n1) in _chunks_512(0, C):
                    for kt in range(NC):
                        nc.tensor.matmul(
                            ps_y[:, n0:n1], attnT[:, kt, t * 128:(t + 1) * 128],
                            wpp[:, kt, n0:n1],
                            start=(kt == 0), stop=(kt == NC - 1))
                y_sb = p3.tile([128, C], F32, tag="y_sb")
                nc.vector.tensor_tensor(y_sb[:], ps_y[:, 0:C],
                                        bb_sb[:, C:2 * C], OP.add)
                nc.sync.dma_start(io["y"][t * 128:(t + 1) * 128, :], y_sb[:])


IO_SPECS = {
    "xT": ([C, T], BF16),
    "wqkv": ([C, 3 * C], BF16),
    "bqkvT": ([128, 3 * C // 128], F32),
    "bias_bcast": ([128, 2 * C], F32),
    "wp": ([C, C], BF16),
    "ones": ([128, 128], F32),
    "triuones16": ([128, 128], BF16),
    "identity16": ([128, 128], BF16),
}
OUT_SPECS = {"y": ([T, C], F32)}


def build_nc():
    from concourse import bacc
    import concourse.tile as tile
    nc = bacc.Bacc("TRN2", target_bir_lowering=False, debug=False,
                   enable_asserts=True, num_devices=N_CORES)
    io = {}
    for name, (shape, dt) in IO_SPECS.items():
        io[name] = nc.dram_tensor(name, shape, dt, kind="ExternalInput").ap()
    for name, (shape, dt) in OUT_SPECS.items():
        io[name] = nc.dram_tensor(name, shape, dt, kind="ExternalOutput").ap()
    with tile.TileContext(nc) as tc:
        _emit_attention(tc, io)
    nc.compile()
    return nc


def host_consts():
    import ml_dtypes
    return {
        "ones": np.ones((128, 128), dtype=np.float32),
        # P^T diag block [k, q]: keep k <= q
        "triuones16": np.triu(np.ones((128, 128))).astype(ml_dtypes.bfloat16),
        "identity16": np.eye(128, dtype=ml_dtypes.bfloat16),
    }


_NC_CACHE = None


def _get_nc():
    global _NC_CACHE
    if _NC_CACHE is None:
        _NC_CACHE = build_nc()
    return _NC_CACHE


def make_in_maps(x, c_attn_kernel, c_attn_bias, c_proj_kernel, c_proj_bias):
    import ml_dtypes
    BF = ml_dtypes.bfloat16
    consts = host_consts()
    wqkv = np.ascontiguousarray(c_attn_kernel).astype(BF)
    bqkv = np.ascontiguousarray(c_attn_bias, dtype=np.float32)
    bqkvT = np.ascontiguousarray(bqkv.reshape(3 * C // 128, 128).T)
    wp = np.ascontiguousarray(c_proj_kernel).astype(BF)
    bp = np.ascontiguousarray(c_proj_bias, dtype=np.float32)
    bias_bcast = np.ascontiguousarray(
        np.tile(np.concatenate([bqkv[2 * C:], bp]), (128, 1)))
    in_maps = []
    for bb in range(N_CORES):
        m = {"xT": np.ascontiguousarray(np.asarray(x[bb]).T).astype(BF),
             "wqkv": wqkv, "bqkvT": bqkvT, "wp": wp,
             "bias_bcast": bias_bcast}
        m.update(consts)
        in_maps.append(m)
    return in_maps


def kernel(x, c_attn_kernel, c_attn_bias, c_proj_kernel, c_proj_bias):
    from concourse.bass_utils import run_bass_kernel_spmd
    x = np.asarray(x)
    assert x.shape == (B, T, C), x.shape
    nc = _get_nc()
    in_maps = make_in_maps(x, c_attn_kernel, c_attn_bias, c_proj_kernel,
                           c_proj_bias)
    res = run_bass_kernel_spmd(nc, in_maps, core_ids=list(range(N_CORES)))
    y = np.stack([res.results[bb]["y"] for bb in range(N_CORES)]).astype(np.float32)
    return y
